# revision 2
# baseline (speedup 1.0000x reference)
"""Trainium2 8-core fused attention kernel (QKV proj + RMSNorm + RoPE + SDPA + out proj).

Sharding: tensor-parallel over heads. Each of the 8 cores computes 2 of the 16
heads end-to-end (QKV projection with its Wqkv column shard, per-head RMSNorm +
RoPE, full softmax attention), then an AllToAll redistributes the per-head
attention outputs so every core holds all 1024 attention channels for 1/8 of
the tokens and applies the full Wout to its token shard.

v3: per-iteration engine balance so the PE never stalls (stalls cost double
via the p-state ramp): every kc iteration splits exp between ACT (head 0,
exact) and DVE (head 1, Schraudolph), both under the PE's 644ns/iter; all
transposes moved to DMA XBAR (dma_start_transpose) freeing PE and the
PSUM-drain copies; softmax normalize on ACT via per-partition scale; batch-1
stage-A and batch-0 out-projection interleaved at kc granularity into the
attention loops; the at_acc two-segment split removed.

Self-contained: hardcodes all shapes from the problem spec.
"""
import os
import sys
import types

import numpy as np
import ml_dtypes

sys.path.insert(0, "/opt/trn_rl_repo")

from concourse import bass, bacc, tile, mybir  # noqa: E402
from concourse.bass_utils import run_bass_kernel_spmd  # noqa: E402

B, N, C, H, D = 2, 4096, 1024, 16, 64
NCORES = 8
TOK = B * N            # 8192 global tokens
NB = N // 128          # 32 token tiles per batch
NMACRO = N // 256      # 16 macro tiles (256 tok) per batch
QTILE = 512
NQT = N // QTILE       # 8 q tiles per batch
KC = N // 128          # 32 key chunks per batch
SHARD = TOK // NCORES  # 1024 tokens per core shard
EPS = 1e-6

F32 = mybir.dt.float32
BF16 = mybir.dt.bfloat16
U16 = mybir.dt.uint16
ALU = mybir.AluOpType
ACTF = mybir.ActivationFunctionType

# Schraudolph exp-via-bits for the DVE: bf16 bitpattern of exp(0.125*s) is
# approx round(A*s + B) as uint16 (error <= +-4.2%, rms 1.8%). Head 1 of every
# batch uses this (50% of elements); the bias cancels between softmax
# numerator and denominator.
SCHRAUD_A = 16.0 / np.log(2.0)
SCHRAUD_B = 16248.75

_CACHE = {}
_LAST_RESULT = None


def _install_profile_shim():
    """trn_boot skips the NTFF hook when antenv.axon_hooks is missing; supply it."""
    try:
        import antenv
        if getattr(antenv, "axon_hooks", None) is not None:
            return
        from trn_agent_boot.trn_boot import _ntff_profile_via_ctypes
        hook = _ntff_profile_via_ctypes("/opt/axon/libaxon_pjrt.so")
        if hook is None:
            return
        mod = types.ModuleType("antenv.axon_hooks")
        state = {"hook": hook}
        mod.get_axon_ntff_profile_hook = lambda: state["hook"]
        mod.set_axon_ntff_profile_hook = lambda h: state.__setitem__("hook", h)
        sys.modules["antenv.axon_hooks"] = mod
        antenv.axon_hooks = mod
    except Exception:
        pass


def _build_graph():
    nc = bacc.Bacc("TRN2", target_bir_lowering=False, debug=False,
                   enable_asserts=True, num_devices=NCORES)

    hsT_d = nc.dram_tensor("hsT", [C, TOK], BF16, kind="ExternalInput")
    wqkv_d = nc.dram_tensor("wqkv", [C, 384], BF16, kind="ExternalInput")
    trigc_d = nc.dram_tensor("trigc", [N, 256], BF16, kind="ExternalInput")
    trigs_d = nc.dram_tensor("trigs", [N, 256], BF16, kind="ExternalInput")
    wout_d = nc.dram_tensor("wout", [C, C], BF16, kind="ExternalInput")
    out_d = nc.dram_tensor("out", [SHARD, C], F32, kind="ExternalOutput")

    with tile.TileContext(nc) as tc:
        with tc.tile_pool(name="const", bufs=1) as constp, \
             tc.tile_pool(name="dram", bufs=1, space="DRAM") as dram:
            # resident weights; per-chunk loads so the first QKV chain starts
            # as soon as chunk 0 lands
            wqkv_sb = constp.tile([128, 8, 384], BF16)
            for cc in range(8):
                nc.sync.dma_start(
                    wqkv_sb[:, cc, :],
                    wqkv_d.ap().rearrange("(a p) n -> p a n", p=128)[:, cc, :])

            a2a_in = [dram.tile([NCORES, 128, SHARD // 2], BF16,
                                  name=f"a2a_in{h}", tag=f"a2a_in{h}") for h in range(2)]
            a2a_out = [dram.tile([NCORES, 128, SHARD // 2], BF16,
                                   name=f"a2a_out{h}", tag=f"a2a_out{h}") for h in range(2)]

            with tc.tile_pool(name="batch", bufs=1) as bp, \
                 tc.tile_pool(name="work", bufs=3) as wp, \
                 tc.tile_pool(name="probsp", bufs=6) as pp, \
                 tc.tile_pool(name="pssc", bufs=2, space="PSUM") as pssc, \
                 tc.tile_pool(name="psav", bufs=2, space="PSUM") as psav:

                qT = [bp.tile([128, N], BF16, name=f"qT{b}", tag=f"qT{b}") for b in range(B)]
                kT = [bp.tile([128, N], BF16, name=f"kT{b}", tag=f"kT{b}") for b in range(B)]
                vsb = [bp.tile([128, NB, 2, 65], BF16, name=f"v{b}", tag=f"v{b}")
                       for b in range(B)]
                # atn2[b][h]: col block pq holds q-chunks (2pq, 2pq+1) of head
                # half h transposed: rows 0:64 = chunk 2pq [d, q], rows 64:128
                # = chunk 2pq+1
                atn2 = [[bp.tile([128, N // 2], BF16, name=f"at{b}{h}",
                                 tag=f"at{b}{h}") for h in range(2)]
                        for b in range(B)]
                for b in range(B):
                    nc.vector.memset(vsb[b][:, :, :, 64:65], 1.0)

                # PE p-state warmup while the first hsT tiles stream in
                warm = pssc.tile([128, 1024], F32, name="warm", tag="pssc")
                wsrc = wqkv_sb[:, 0, 0:128]
                for _ in range(144):
                    nc.tensor.matmul(warm[:, 0:128], lhsT=wsrc, rhs=wsrc,
                                     start=True, stop=True)

                # ---------------- Stage A: QKV + RMSNorm + RoPE ----------------
                def emit_A_load(b, mt, sp):
                    hs_t = []
                    for cc in range(8):
                        t = sp.tile([128, 256], BF16, name=f"hs{cc}", tag=f"hs{cc}")
                        nc.sync.dma_start(
                            t[:], hsT_d.ap()[cc * 128:(cc + 1) * 128,
                                             b * N + mt * 256: b * N + (mt + 1) * 256])
                        hs_t.append(t)
                    trigC = sp.tile([128, 2, 256], BF16, name="trigC", tag="trigC")
                    trigS = sp.tile([128, 2, 256], BF16, name="trigS", tag="trigS")
                    for dst, dt_ in ((trigC, trigc_d), (trigS, trigs_d)):
                        nc.sync.dma_start(
                            dst[:], dt_.ap()[mt * 256:(mt + 1) * 256, :]
                            .rearrange("(s p) d -> p s d", p=128))
                    return hs_t, trigC, trigS

                def emit_A_sub(b, mt, sub, ld):
                    hs_t, trigC, trigS = ld
                    tt = mt * 2 + sub
                    # q+k chain in bank 0, v chain in bank 1: alternating
                    # banks lets the accumulating writes pipeline
                    ps_qkv = pssc.tile([128, 1024], F32, name="ps_qkv", tag="pssc")
                    for cc in range(8):
                        lhs = hs_t[cc][:, sub * 128:(sub + 1) * 128]
                        nc.tensor.matmul(
                            ps_qkv[:, 0:256], lhsT=lhs,
                            rhs=wqkv_sb[:, cc, 0:256],
                            start=(cc == 0), stop=(cc == 7))
                        nc.tensor.matmul(
                            ps_qkv[:, 512:640], lhsT=lhs,
                            rhs=wqkv_sb[:, cc, 256:384],
                            start=(cc == 0), stop=(cc == 7))

                    # ACT: psum drains (bf16) + squares; DVE: reduce + rsqrt
                    # newton + RoPE (bf16 ops run in the DVE 2x port mode)
                    qk_sb = wp.tile([128, 256], BF16, name="qk_sb", tag="qk_sb",
                                    bufs=4)
                    nc.scalar.copy(qk_sb[:], ps_qkv[:, 0:256])
                    nc.scalar.copy(
                        vsb[b][:, tt, :, 0:64],
                        ps_qkv[:, 512:640].rearrange("p (h d) -> p h d", h=2))
                    sq = wp.tile([128, 256], BF16, name="sq", tag="sq", bufs=4)
                    nc.scalar.square(sq[:], qk_sb[:])
                    # sumsq for (q h0, q h1, k h0, k h1) -> [128, 4]
                    ssq4 = wp.tile([128, 4], F32, name="ssq4", tag="ssq4", bufs=4)
                    nc.vector.tensor_reduce(
                        ssq4[:], sq[:].rearrange("p (a e) -> p a e", a=4),
                        axis=mybir.AxisListType.X, op=ALU.add)
                    # rinv = 8/sqrt(ssq): bit-trick seed + 1 Newton step
                    yv = wp.tile([128, 4], F32, name="yv", tag="yv")
                    with nc.allow_low_precision(reason="rsqrt newton seed"):
                        nc.vector.tensor_scalar(
                            out=yv[:].bitcast(mybir.dt.int32),
                            in0=ssq4[:].bitcast(mybir.dt.int32),
                            scalar1=1, scalar2=None, op0=ALU.arith_shift_right)
                        nc.vector.tensor_scalar(
                            out=yv[:].bitcast(mybir.dt.int32),
                            in0=yv[:].bitcast(mybir.dt.int32),
                            scalar1=-1, scalar2=0x5F3759DF,
                            op0=ALU.mult, op1=ALU.add)
                    tn = wp.tile([128, 4], F32, name="tn", tag="tn")
                    nc.vector.tensor_mul(tn[:], yv[:], yv[:])
                    nc.vector.tensor_mul(tn[:], tn[:], ssq4[:])
                    nc.vector.tensor_scalar(out=tn[:], in0=tn[:],
                                            scalar1=-4.0, scalar2=12.0,
                                            op0=ALU.mult, op1=ALU.add)
                    nc.vector.tensor_mul(yv[:], yv[:], tn[:])
                    # RoPE on the unnormalized values (bf16, 2x mode); the
                    # per-group rinv commutes with the rotation, applied last
                    d_qk = wp.tile([128, 256], BF16, name="d_qk", tag="d_qk", bufs=3)
                    nc.vector.tensor_mul(d_qk[:], qk_sb[:], trigC[:, sub, :])
                    trot = wp.tile([128, 256], BF16, name="trot", tag="trot", bufs=3)
                    v4 = qk_sb[:].rearrange("p (a e) -> p a e", a=8)
                    s4 = trigS[:, sub, :].rearrange("p (a e) -> p a e", a=8)
                    t4 = trot[:].rearrange("p (a e) -> p a e", a=8)
                    nc.vector.tensor_mul(t4[:, 0:8:2, :], v4[:, 1:8:2, :],
                                         s4[:, 0:8:2, :])
                    nc.vector.tensor_mul(t4[:, 1:8:2, :], v4[:, 0:8:2, :],
                                         s4[:, 1:8:2, :])
                    rope = wp.tile([128, 256], BF16, name="rope", tag="rope", bufs=3)
                    nc.vector.tensor_add(rope[:], d_qk[:], trot[:])
                    d_bf = wp.tile([128, 256], BF16, name="d_bf", tag="d_bf", bufs=4)
                    nc.vector.tensor_tensor(
                        out=d_bf[:].rearrange("p (a e) -> p a e", a=4),
                        in0=rope[:].rearrange("p (a e) -> p a e", a=4),
                        in1=yv[:].unsqueeze(2).broadcast_to([128, 4, 64]),
                        op=ALU.mult)
                    # transposes via DMA XBAR: frees PE and the drain copies
                    nc.sync.dma_start_transpose(
                        qT[b][:, tt * 128:(tt + 1) * 128], d_bf[:, 0:128])
                    nc.sync.dma_start_transpose(
                        kT[b][:, tt * 128:(tt + 1) * 128], d_bf[:, 128:256])

                def emit_A(b, mt, sp):
                    ld = emit_A_load(b, mt, sp)
                    emit_A_sub(b, mt, 0, ld)
                    emit_A_sub(b, mt, 1, ld)

                # ---------------- Stage B: attention --------------------------
                # Flipped AV: probs chunk [128kc, 128q] stationary, [v|1] moving.
                # at_ps[:, u, 0:65] (u = hh*4+j) accumulates [128q, 64d | denom].

                def emit_B(b, qt, filler=None):
                    at_ps = psav.tile([128, 8, 128], F32, name="at_ps", tag="psav")
                    prevs = []

                    def av_half(pkc, ppr, hh, stop):
                        # start=True clears accumulate bits for the WHOLE psum
                        # bank, so only the first group per bank (u=0, u=4) may
                        # set it
                        for j in range(4):
                            u = hh * 4 + j
                            nc.tensor.matmul(
                                at_ps[:, u, 0:65],
                                lhsT=ppr[:, hh * QTILE + j * 128:
                                         hh * QTILE + (j + 1) * 128],
                                rhs=vsb[b][:, pkc, hh, :],
                                start=(pkc == 0 and j == 0),
                                stop=stop,
                                skip_group_check=(j != 0))

                    def scores(kc, hh):
                        nc.tensor.matmul(
                            ps_s[:, hh * QTILE:(hh + 1) * QTILE],
                            lhsT=kT[b][64 * hh:64 * (hh + 1),
                                       kc * 128:(kc + 1) * 128],
                            rhs=qT[b][64 * hh:64 * (hh + 1),
                                      qt * QTILE:(qt + 1) * QTILE],
                            start=True, stop=True)

                    for kc in range(KC):
                        ps_s = pssc.tile([128, 2 * QTILE], F32, name="ps_s",
                                         tag="pssc")
                        scores(kc, 0)
                        scores(kc, 1)
                        pr = pp.tile([128, 2 * QTILE], BF16, name="pr", tag="pr",
                                     bufs=6)
                        # head 0 on ACT (exact), head 1 via DVE Schraudolph
                        # (different psum banks -> legal parallel access)
                        nc.scalar.activation(pr[:, 0:QTILE], ps_s[:, 0:QTILE],
                                             ACTF.Exp, bias=0.0, scale=0.125)
                        with nc.allow_low_precision(reason="schraudolph exp"):
                            nc.vector.tensor_scalar(
                                out=pr[:, QTILE:2 * QTILE].bitcast(U16),
                                in0=ps_s[:, QTILE:2 * QTILE],
                                scalar1=float(SCHRAUD_A),
                                scalar2=float(SCHRAUD_B),
                                op0=ALU.mult, op1=ALU.add)
                        if len(prevs) == 2:
                            ppkc, pppr = prevs.pop(0)
                            av_half(ppkc, pppr, 0, False)
                            av_half(ppkc, pppr, 1, False)
                        if filler is not None:
                            filler(kc)
                        prevs.append((kc, pr))

                    # drain the 2-deep pipeline tail
                    ppkc, pppr = prevs.pop(0)
                    av_half(ppkc, pppr, 0, False)
                    av_half(ppkc, pppr, 1, False)
                    ppkc, pppr = prevs.pop(0)
                    av_half(ppkc, pppr, 0, True)
                    av_half(ppkc, pppr, 1, True)

                    # normalize: batched reciprocal of the 8 denominators on
                    # DVE, then ACT copy-with-per-partition-scale per group
                    rcp8 = wp.tile([128, 8], F32, name="rcp8", tag="rcp8", bufs=4)
                    nc.vector.reciprocal_approx_fast(
                        out=rcp8[:], in_=at_ps[:, :, 64:65].rearrange(
                            "p u c -> p (u c)"))
                    dsbs = []
                    for p in range(4):
                        dsb = wp.tile([128, 128], BF16, name="dsb", tag="dsb",
                                      bufs=8)
                        dsbs.append(dsb)
                    for u in range(8):
                        nc.scalar.activation(
                            dsbs[u // 2][:, (u % 2) * 64:(u % 2) * 64 + 64],
                            at_ps[:, u, 0:64], ACTF.Copy,
                            bias=0.0, scale=rcp8[:, u:u + 1])
                    # transpose pairs into atn2 via DMA XBAR, then stage this
                    # qtile's a2a block (dest core == qt)
                    for p in range(4):
                        hh, lp = divmod(p, 2)
                        pq = 2 * qt + lp
                        nc.sync.dma_start_transpose(
                            atn2[b][hh][:, pq * 128:(pq + 1) * 128], dsbs[p][:])
                    for hh in range(2):
                        for lp in range(2):
                            pq = 2 * qt + lp
                            src = atn2[b][hh][:, pq * 128:(pq + 1) * 128]
                            for rh in range(2):
                                nc.sync.dma_start(
                                    a2a_in[b][qt, hh * 64:(hh + 1) * 64,
                                              lp * 256 + rh * 128:
                                              lp * 256 + (rh + 1) * 128],
                                    src[rh * 64:(rh + 1) * 64, :])

                # ---- phase 1: all of batch-0 stage A -------------------------
                with tc.tile_pool(name="stream", bufs=6) as sp:
                    for mt in range(NMACRO):
                        emit_A(0, mt, sp)

                    # ---- phase 2: batch-0 attention + batch-1 stage A --------
                    for qt in range(NQT):
                        mtA, mtB = 2 * qt, 2 * qt + 1
                        lds = {}

                        def filler(kc, mtA=mtA, mtB=mtB, lds=lds):
                            if kc == 0:
                                lds["A"] = emit_A_load(1, mtA, sp)
                            elif kc == 6:
                                emit_A_sub(1, mtA, 0, lds["A"])
                            elif kc == 12:
                                emit_A_sub(1, mtA, 1, lds["A"])
                                lds["B"] = emit_A_load(1, mtB, sp)
                            elif kc == 18:
                                emit_A_sub(1, mtB, 0, lds["B"])
                            elif kc == 24:
                                emit_A_sub(1, mtB, 1, lds["B"])

                        emit_B(0, qt, filler=filler)

                nc.gpsimd.collective_compute(
                    "AllToAll", ALU.bypass,
                    ins=[a2a_in[0][:].opt()], outs=[a2a_out[0][:].opt()],
                    replica_groups=[list(range(NCORES))])

                # ---- phase 3: batch-1 attention + batch-0 out projection ----
                with tc.tile_pool(name="cstage", bufs=1) as cp, \
                     tc.tile_pool(name="cwork", bufs=2) as cw:
                    wout_sb = cp.tile([128, 8, C], BF16)
                    nc.sync.dma_start(
                        wout_sb[:], wout_d.ap().rearrange("(a p) n -> p a n", p=128))
                    atf = cp.tile([128, 8, SHARD], BF16)
                    nc.sync.dma_start(atf[:, :, 0:512],
                                      a2a_out[0][:].transpose([1, 0, 2]))

                    osts = {}

                    def emit_C(ttk, half, drain_eng):
                        # atomic unit: 16 matmuls in two bank-alternating
                        # half-chains + psum drain (shares pssc ring)
                        if half == 0:
                            osts[ttk] = cw.tile([128, C], F32, name="ostage",
                                                tag="ostage")
                        ost = osts[ttk]
                        ps_o = pssc.tile([128, 1024], F32, name="ps_o", tag="pssc")
                        for cc in range(8):
                            lhs = atf[:, cc, ttk * 128:(ttk + 1) * 128]
                            nc.tensor.matmul(
                                ps_o[:, 0:256], lhsT=lhs,
                                rhs=wout_sb[:, cc, half * 512:half * 512 + 256],
                                start=(cc == 0), stop=(cc == 7))
                            nc.tensor.matmul(
                                ps_o[:, 512:768], lhsT=lhs,
                                rhs=wout_sb[:, cc, half * 512 + 256:
                                            half * 512 + 512],
                                start=(cc == 0), stop=(cc == 7))
                        dst = ost[:, half * 512:(half + 1) * 512] \
                            .rearrange("p (a b) -> p a b", a=2)
                        src = ps_o[:].rearrange("p (a b) -> p a b", a=2)[:, :, 0:256]
                        if drain_eng == "act":
                            nc.scalar.copy(dst, src)
                        else:
                            nc.vector.tensor_copy(dst, src)
                        nc.sync.dma_start(
                            out_d.ap()[ttk * 128:(ttk + 1) * 128,
                                       half * 512:(half + 1) * 512],
                            ost[:, half * 512:(half + 1) * 512])

                    # interleave batch-0 out-proj units into batch-1 attention
                    # (atf half 0 lands ~2 qt into the phase)
                    cunits = [(t, h) for t in range(4) for h in range(2)]

                    for qt in range(NQT):
                        def filler3(kc, qt=qt):
                            if kc == 16 and qt >= 2 and cunits:
                                emit_C(*cunits.pop(0), drain_eng="vec")
                        emit_B(1, qt, filler=filler3)

                    nc.gpsimd.collective_compute(
                        "AllToAll", ALU.bypass,
                        ins=[a2a_in[1][:].opt()], outs=[a2a_out[1][:].opt()],
                        replica_groups=[list(range(NCORES))])

                    # leftover batch-0 units overlap the collective's flight
                    while cunits:
                        emit_C(*cunits.pop(0), drain_eng="act")
                    nc.sync.dma_start(atf[:, :, 512:1024],
                                      a2a_out[1][:].transpose([1, 0, 2]))
                    for ttk in range(4, 8):
                        for half in range(2):
                            emit_C(ttk, half, drain_eng="act")

    nc.compile()
    return nc


def _fold_sin(sin, g):
    out = np.empty_like(sin)
    out[:, :32] = -sin[:, :32] * g[32:]
    out[:, 32:] = sin[:, 32:] * g[:32]
    return out


def kernel(hidden_states, cos, sin, Wqkv, Wout, gq, gk):
    global _LAST_RESULT
    _install_profile_shim()

    hidden_states = np.asarray(hidden_states, dtype=np.float32)
    cos = np.asarray(cos, dtype=np.float32)
    sin = np.asarray(sin, dtype=np.float32)
    Wqkv = np.asarray(Wqkv, dtype=np.float32)
    Wout = np.asarray(Wout, dtype=np.float32)
    gq = np.asarray(gq, dtype=np.float32)
    gk = np.asarray(gk, dtype=np.float32)

    if "nc" not in _CACHE:
        _CACHE["nc"] = _build_graph()
    nc = _CACHE["nc"]

    hsT = np.ascontiguousarray(hidden_states.reshape(TOK, C).T).astype(ml_dtypes.bfloat16)
    cosq = cos * gq[None, :]
    sinq = _fold_sin(sin, gq)
    cosk = cos * gk[None, :]
    sink = _fold_sin(sin, gk)
    trigc = np.concatenate([cosq, cosq, cosk, cosk], axis=1).astype(ml_dtypes.bfloat16)
    trigs = np.concatenate([sinq, sinq, sink, sink], axis=1).astype(ml_dtypes.bfloat16)
    wout_bf = Wout.astype(ml_dtypes.bfloat16)

    in_maps = []
    for c in range(NCORES):
        wq = Wqkv[:, c * 128:(c + 1) * 128]
        wk = Wqkv[:, C + c * 128:C + (c + 1) * 128]
        wv = Wqkv[:, 2 * C + c * 128:2 * C + (c + 1) * 128]
        wqkv_loc = np.ascontiguousarray(
            np.concatenate([wq, wk, wv], axis=1)).astype(ml_dtypes.bfloat16)
        in_maps.append({
            "hsT": hsT, "wqkv": wqkv_loc, "trigc": trigc, "trigs": trigs,
            "wout": wout_bf,
        })

    trace = bool(os.environ.get("BASS_TRACE"))
    res = run_bass_kernel_spmd(nc, in_maps, core_ids=list(range(NCORES)), trace=trace)
    _LAST_RESULT = res

    full = np.empty((B, N, C), dtype=np.float32)
    for c in range(NCORES):
        o = res.results[c]["out"]
        for b in range(B):
            full[b, c * 512:(c + 1) * 512, :] = o[b * 512:(b + 1) * 512]
    return full


# revision 3
# speedup vs baseline: 1.2716x; 1.2716x over previous
"""Trainium2 8-core fused attention kernel (QKV proj + RMSNorm + RoPE + SDPA + out proj).

Sharding: tensor-parallel over heads. Each of the 8 cores computes 2 of the 16
heads end-to-end (QKV projection with its Wqkv column shard, per-head RMSNorm +
RoPE, full softmax attention), then an AllToAll redistributes the per-head
attention outputs so every core holds all 1024 attention channels for 1/8 of
the tokens and applies the full Wout to its token shard.

v3.1: engine-balanced pipeline. Exp alternates whole-tile between ACT (even
kc, exact) and DVE (odd kc, Schraudolph) so each engine pays instruction
overhead half as often and both stay under the PE's per-iteration budget;
batch-1 stage-A work is injected into batch-0's attention loop in small
single-engine pieces to avoid cross-engine queue bubbles; softmax normalize
runs on ACT via per-partition scale; drain transposes use the DMA XBAR;
batch-0 out-projection interleaves into batch-1 attention.

Self-contained: hardcodes all shapes from the problem spec.
"""
import os
import sys
import types

import numpy as np
import ml_dtypes

sys.path.insert(0, "/opt/trn_rl_repo")

from concourse import bass, bacc, tile, mybir  # noqa: E402
from concourse.bass_utils import run_bass_kernel_spmd  # noqa: E402
from concourse.masks import make_identity  # noqa: E402

B, N, C, H, D = 2, 4096, 1024, 16, 64
NCORES = 8
TOK = B * N            # 8192 global tokens
NB = N // 128          # 32 token tiles per batch
NMACRO = N // 256      # 16 macro tiles (256 tok) per batch
QTILE = 512
NQT = N // QTILE       # 8 q tiles per batch
KC = N // 128          # 32 key chunks per batch
SHARD = TOK // NCORES  # 1024 tokens per core shard
EPS = 1e-6

F32 = mybir.dt.float32
BF16 = mybir.dt.bfloat16
U16 = mybir.dt.uint16
ALU = mybir.AluOpType
ACTF = mybir.ActivationFunctionType

# Schraudolph exp-via-bits for the DVE: bf16 bitpattern of exp(0.125*s) is
# approx round(A*s + B) as uint16 (error <= +-4.2%, rms 1.8%). Odd key-chunks
# use this (50% of elements); the bias cancels between softmax numerator and
# denominator.
SCHRAUD_A = 16.0 / np.log(2.0)
SCHRAUD_B = 16248.75

_CACHE = {}
_LAST_RESULT = None


def _install_profile_shim():
    """trn_boot skips the NTFF hook when antenv.axon_hooks is missing; supply it."""
    try:
        import antenv
        if getattr(antenv, "axon_hooks", None) is not None:
            return
        from trn_agent_boot.trn_boot import _ntff_profile_via_ctypes
        hook = _ntff_profile_via_ctypes("/opt/axon/libaxon_pjrt.so")
        if hook is None:
            return
        mod = types.ModuleType("antenv.axon_hooks")
        state = {"hook": hook}
        mod.get_axon_ntff_profile_hook = lambda: state["hook"]
        mod.set_axon_ntff_profile_hook = lambda h: state.__setitem__("hook", h)
        sys.modules["antenv.axon_hooks"] = mod
        antenv.axon_hooks = mod
    except Exception:
        pass


def _build_graph():
    nc = bacc.Bacc("TRN2", target_bir_lowering=False, debug=False,
                   enable_asserts=True, num_devices=NCORES)

    hsT_d = nc.dram_tensor("hsT", [C, TOK], BF16, kind="ExternalInput")
    wqkv_d = nc.dram_tensor("wqkv", [C, 384], BF16, kind="ExternalInput")
    trigc_d = nc.dram_tensor("trigc", [N, 256], BF16, kind="ExternalInput")
    trigs_d = nc.dram_tensor("trigs", [N, 256], BF16, kind="ExternalInput")
    wout_d = nc.dram_tensor("wout", [C, C], BF16, kind="ExternalInput")
    out_d = nc.dram_tensor("out", [SHARD, C], F32, kind="ExternalOutput")

    with tile.TileContext(nc) as tc:
        with tc.tile_pool(name="const", bufs=1) as constp, \
             tc.tile_pool(name="dram", bufs=1, space="DRAM") as dram:
            # resident weights; per-chunk loads so the first QKV chain starts
            # as soon as chunk 0 lands
            wqkv_sb = constp.tile([128, 8, 384], BF16)
            for cc in range(8):
                nc.sync.dma_start(
                    wqkv_sb[:, cc, :],
                    wqkv_d.ap().rearrange("(a p) n -> p a n", p=128)[:, cc, :])
            ident = constp.tile([128, 128], BF16)
            make_identity(nc, ident[:])

            a2a_in = [dram.tile([NCORES, 128, SHARD // 2], BF16,
                                  name=f"a2a_in{h}", tag=f"a2a_in{h}") for h in range(2)]
            a2a_out = [dram.tile([NCORES, 128, SHARD // 2], BF16,
                                   name=f"a2a_out{h}", tag=f"a2a_out{h}") for h in range(2)]

            with tc.tile_pool(name="batch", bufs=1) as bp, \
                 tc.tile_pool(name="work", bufs=3) as wp, \
                 tc.tile_pool(name="probsp", bufs=6) as pp, \
                 tc.tile_pool(name="pssc", bufs=2, space="PSUM") as pssc:

                qT = [bp.tile([128, N], BF16, name=f"qT{b}", tag=f"qT{b}") for b in range(B)]
                kT = [bp.tile([128, N], BF16, name=f"kT{b}", tag=f"kT{b}") for b in range(B)]
                vsb = [bp.tile([128, NB, 2, 65], BF16, name=f"v{b}", tag=f"v{b}")
                       for b in range(B)]
                # atn2[b][h]: col block pq holds q-chunks (2pq, 2pq+1) of head
                # half h transposed: rows 0:64 = chunk 2pq [d, q], rows 64:128
                # = chunk 2pq+1
                atn2 = [[bp.tile([128, N // 2], BF16, name=f"at{b}{h}",
                                 tag=f"at{b}{h}") for h in range(2)]
                        for b in range(B)]
                for b in range(B):
                    nc.vector.memset(vsb[b][:, :, :, 64:65], 1.0)

                # PE p-state warmup while the first hsT tiles stream in
                warm = pssc.tile([128, 1024], F32, name="warm", tag="pssc")
                for _ in range(144):
                    nc.tensor.matmul(warm[:, 0:128], lhsT=ident[:], rhs=ident[:],
                                     start=True, stop=True)

                # ---------------- Stage A: QKV + RMSNorm + RoPE ----------------
                def emit_A_load(b, mt, sp):
                    hs_t = []
                    for cc in range(8):
                        t = sp.tile([128, 256], BF16, name=f"hs{cc}", tag=f"hs{cc}")
                        nc.sync.dma_start(
                            t[:], hsT_d.ap()[cc * 128:(cc + 1) * 128,
                                             b * N + mt * 256: b * N + (mt + 1) * 256])
                        hs_t.append(t)
                    trigC = sp.tile([128, 2, 256], BF16, name="trigC", tag="trigC")
                    trigS = sp.tile([128, 2, 256], BF16, name="trigS", tag="trigS")
                    for dst, dt_ in ((trigC, trigc_d), (trigS, trigs_d)):
                        nc.sync.dma_start(
                            dst[:], dt_.ap()[mt * 256:(mt + 1) * 256, :]
                            .rearrange("(s p) d -> p s d", p=128))
                    return hs_t, trigC, trigS

                def a_piece1(b, mt, sub, ld, st):
                    """PE QKV chains + ACT psum drains + square."""
                    hs_t, trigC, trigS = ld
                    tt = mt * 2 + sub
                    ps_qkv = pssc.tile([128, 1024], F32, name="ps_qkv", tag="pssc")
                    for cc in range(8):
                        lhs = hs_t[cc][:, sub * 128:(sub + 1) * 128]
                        nc.tensor.matmul(
                            ps_qkv[:, 0:256], lhsT=lhs,
                            rhs=wqkv_sb[:, cc, 0:256],
                            start=(cc == 0), stop=(cc == 7))
                        nc.tensor.matmul(
                            ps_qkv[:, 512:640], lhsT=lhs,
                            rhs=wqkv_sb[:, cc, 256:384],
                            start=(cc == 0), stop=(cc == 7))
                    qk_sb = wp.tile([128, 256], BF16, name="qk_sb", tag="qk_sb",
                                    bufs=5)
                    nc.scalar.copy(qk_sb[:], ps_qkv[:, 0:256])
                    nc.scalar.copy(
                        vsb[b][:, tt, :, 0:64],
                        ps_qkv[:, 512:640].rearrange("p (h d) -> p h d", h=2))
                    sq = wp.tile([128, 256], BF16, name="sq", tag="sq", bufs=5)
                    nc.scalar.square(sq[:], qk_sb[:])
                    st["qk_sb"], st["sq"] = qk_sb, sq
                    st["trigC"], st["trigS"] = trigC, trigS

                def a_piece2(st):
                    """DVE sumsq reduce + rsqrt newton (rinv = 8/sqrt(ssq))."""
                    sq = st["sq"]
                    ssq4 = wp.tile([128, 4], F32, name="ssq4", tag="ssq4", bufs=4)
                    nc.vector.tensor_reduce(
                        ssq4[:], sq[:].rearrange("p (a e) -> p a e", a=4),
                        axis=mybir.AxisListType.X, op=ALU.add)
                    yv = wp.tile([128, 4], F32, name="yv", tag="yv", bufs=4)
                    with nc.allow_low_precision(reason="rsqrt newton seed"):
                        nc.vector.tensor_scalar(
                            out=yv[:].bitcast(mybir.dt.int32),
                            in0=ssq4[:].bitcast(mybir.dt.int32),
                            scalar1=1, scalar2=None, op0=ALU.arith_shift_right)
                        nc.vector.tensor_scalar(
                            out=yv[:].bitcast(mybir.dt.int32),
                            in0=yv[:].bitcast(mybir.dt.int32),
                            scalar1=-1, scalar2=0x5F3759DF,
                            op0=ALU.mult, op1=ALU.add)
                    tn = wp.tile([128, 4], F32, name="tn", tag="tn", bufs=4)
                    nc.vector.tensor_mul(tn[:], yv[:], yv[:])
                    nc.vector.tensor_mul(tn[:], tn[:], ssq4[:])
                    nc.vector.tensor_scalar(out=tn[:], in0=tn[:],
                                            scalar1=-4.0, scalar2=12.0,
                                            op0=ALU.mult, op1=ALU.add)
                    nc.vector.tensor_mul(yv[:], yv[:], tn[:])
                    st["yv"] = yv

                def a_piece3(sub, st):
                    """DVE RoPE (bf16 2x) + apply rinv last (it commutes)."""
                    qk_sb, yv = st["qk_sb"], st["yv"]
                    trigC, trigS = st["trigC"], st["trigS"]
                    d_qk = wp.tile([128, 256], BF16, name="d_qk", tag="d_qk", bufs=3)
                    nc.vector.tensor_mul(d_qk[:], qk_sb[:], trigC[:, sub, :])
                    trot = wp.tile([128, 256], BF16, name="trot", tag="trot", bufs=3)
                    v4 = qk_sb[:].rearrange("p (a e) -> p a e", a=8)
                    s4 = trigS[:, sub, :].rearrange("p (a e) -> p a e", a=8)
                    t4 = trot[:].rearrange("p (a e) -> p a e", a=8)
                    nc.vector.tensor_mul(t4[:, 0:8:2, :], v4[:, 1:8:2, :],
                                         s4[:, 0:8:2, :])
                    nc.vector.tensor_mul(t4[:, 1:8:2, :], v4[:, 0:8:2, :],
                                         s4[:, 1:8:2, :])
                    rope = wp.tile([128, 256], BF16, name="rope", tag="rope", bufs=3)
                    nc.vector.tensor_add(rope[:], d_qk[:], trot[:])
                    d_bf = wp.tile([128, 256], BF16, name="d_bf", tag="d_bf", bufs=6)
                    nc.vector.tensor_tensor(
                        out=d_bf[:].rearrange("p (a e) -> p a e", a=4),
                        in0=rope[:].rearrange("p (a e) -> p a e", a=4),
                        in1=yv[:].unsqueeze(2).broadcast_to([128, 4, 64]),
                        op=ALU.mult)
                    st["d_bf"] = d_bf

                def a_piece4_xbar(b, mt, sub, st):
                    """qT/kT transposes via DMA XBAR (SP queue)."""
                    tt = mt * 2 + sub
                    d_bf = st["d_bf"]
                    nc.sync.dma_start_transpose(
                        qT[b][:, tt * 128:(tt + 1) * 128], d_bf[:, 0:128])
                    nc.sync.dma_start_transpose(
                        kT[b][:, tt * 128:(tt + 1) * 128], d_bf[:, 128:256])

                # ---------------- Stage B: attention --------------------------
                # Flipped AV: probs chunk [128kc, 128q] stationary, [v|1] moving.
                # at_ps[:, u, 0:65] (u = hh*4+j) accumulates [128q, 64d | denom].

                def emit_B(b, qt, psav, filler=None):
                    at_ps = psav.tile([128, 8, 128], F32, name="at_ps", tag="psav")
                    prevs = []

                    def av_half(pkc, ppr, hh, stop):
                        # start=True clears accumulate bits for the WHOLE psum
                        # bank, so only the first group per bank (u=0, u=4) may
                        # set it
                        for j in range(4):
                            u = hh * 4 + j
                            nc.tensor.matmul(
                                at_ps[:, u, 0:65],
                                lhsT=ppr[:, hh * QTILE + j * 128:
                                         hh * QTILE + (j + 1) * 128],
                                rhs=vsb[b][:, pkc, hh, :],
                                start=(pkc == 0 and j == 0),
                                stop=stop,
                                skip_group_check=(j != 0))

                    def scores(kc, hh):
                        nc.tensor.matmul(
                            ps_s[:, hh * QTILE:(hh + 1) * QTILE],
                            lhsT=kT[b][64 * hh:64 * (hh + 1),
                                       kc * 128:(kc + 1) * 128],
                            rhs=qT[b][64 * hh:64 * (hh + 1),
                                      qt * QTILE:(qt + 1) * QTILE],
                            start=True, stop=True)

                    for kc in range(KC):
                        ps_s = pssc.tile([128, 2 * QTILE], F32, name="ps_s",
                                         tag="pssc")
                        scores(kc, 0)
                        scores(kc, 1)
                        pr = pp.tile([128, 2 * QTILE], BF16, name="pr", tag="pr",
                                     bufs=6)
                        # whole-tile exp, alternating engines: even kc on ACT
                        # (exact), odd kc on DVE (Schraudolph) — halves the
                        # per-instruction overhead on each engine
                        if kc % 2 == 0:
                            nc.scalar.activation(pr[:], ps_s[:], ACTF.Exp,
                                                 bias=0.0, scale=0.125)
                        else:
                            with nc.allow_low_precision(reason="schraudolph exp"):
                                nc.vector.tensor_scalar(
                                    out=pr[:].bitcast(U16),
                                    in0=ps_s[:],
                                    scalar1=float(SCHRAUD_A),
                                    scalar2=float(SCHRAUD_B),
                                    op0=ALU.mult, op1=ALU.add)
                        if len(prevs) == 2:
                            ppkc, pppr = prevs.pop(0)
                            av_half(ppkc, pppr, 0, False)
                            av_half(ppkc, pppr, 1, False)
                        if filler is not None:
                            filler(kc)
                        prevs.append((kc, pr))

                    # drain the 2-deep pipeline tail
                    ppkc, pppr = prevs.pop(0)
                    av_half(ppkc, pppr, 0, False)
                    av_half(ppkc, pppr, 1, False)
                    ppkc, pppr = prevs.pop(0)
                    av_half(ppkc, pppr, 0, True)
                    av_half(ppkc, pppr, 1, True)

                    # normalize: batched reciprocal of the 8 denominators on
                    # DVE, then ACT copy-with-per-partition-scale per group
                    rcp8 = wp.tile([128, 8], F32, name="rcp8", tag="rcp8", bufs=4)
                    nc.vector.reciprocal_approx_fast(
                        out=rcp8[:], in_=at_ps[:, :, 64:65].rearrange(
                            "p u c -> p (u c)"))
                    dsbs = []
                    for p in range(4):
                        dsb = wp.tile([128, 128], BF16, name="dsb", tag="dsb",
                                      bufs=8)
                        dsbs.append(dsb)
                    for u in range(8):
                        nc.scalar.activation(
                            dsbs[u // 2][:, (u % 2) * 64:(u % 2) * 64 + 64],
                            at_ps[:, u, 0:64], ACTF.Copy,
                            bias=0.0, scale=rcp8[:, u:u + 1])
                    # transpose pairs into atn2 via DMA XBAR, then stage this
                    # qtile's a2a block (dest core == qt)
                    for p in range(4):
                        hh, lp = divmod(p, 2)
                        pq = 2 * qt + lp
                        nc.sync.dma_start_transpose(
                            atn2[b][hh][:, pq * 128:(pq + 1) * 128], dsbs[p][:])
                    for hh in range(2):
                        for lp in range(2):
                            pq = 2 * qt + lp
                            src = atn2[b][hh][:, pq * 128:(pq + 1) * 128]
                            for rh in range(2):
                                nc.sync.dma_start(
                                    a2a_in[b][qt, hh * 64:(hh + 1) * 64,
                                              lp * 256 + rh * 128:
                                              lp * 256 + (rh + 1) * 128],
                                    src[rh * 64:(rh + 1) * 64, :])

                # ---- phase 1: all of batch-0 stage A (PE transposes) --------
                with tc.tile_pool(name="stream", bufs=6) as sp:
                    with tc.tile_pool(name="psT", bufs=2, space="PSUM") as psT:
                        for mt in range(NMACRO):
                            ld = emit_A_load(0, mt, sp)
                            for sub in range(2):
                                st = {}
                                a_piece1(0, mt, sub, ld, st)
                                a_piece2(st)
                                a_piece3(sub, st)
                                tt = mt * 2 + sub
                                d_bf = st["d_bf"]
                                for half, dst in ((0, qT[0]), (1, kT[0])):
                                    ps_t = psT.tile([128, 128], BF16,
                                                    name="ps_t", tag="pst")
                                    nc.tensor.transpose(
                                        ps_t[:],
                                        d_bf[:, half * 128:(half + 1) * 128],
                                        ident[:])
                                    nc.scalar.copy(
                                        dst[:, tt * 128:(tt + 1) * 128], ps_t[:])

                    # ---- phase 2: batch-0 attention + batch-1 stage A --------
                    with tc.tile_pool(name="psav", bufs=2, space="PSUM") as psav:
                        for qt in range(NQT):
                            mtA, mtB = 2 * qt, 2 * qt + 1
                            ctx = {"lds": {}, "sts": {}}

                            def filler(kc, qt=qt, mtA=mtA, mtB=mtB, ctx=ctx):
                                lds, sts = ctx["lds"], ctx["sts"]
                                # 4 subs x 4 pieces, spread over the 32 kc
                                # iterations; each piece is single-engine
                                for s in range(4):
                                    mt = mtA if s < 2 else mtB
                                    sub = s % 2
                                    base = 1 + s * 7
                                    if kc == base - 1 and sub == 0:
                                        lds[mt] = emit_A_load(1, mt, sp)
                                    elif kc == base:
                                        sts[s] = {}
                                        a_piece1(1, mt, sub, lds[mt], sts[s])
                                    elif kc == base + 2:
                                        a_piece2(sts[s])
                                    elif kc == base + 4:
                                        a_piece3(sub, sts[s])
                                    elif kc == base + 6:
                                        a_piece4_xbar(1, mt, sub, sts[s])

                            emit_B(0, qt, psav, filler=filler)

                        nc.gpsimd.collective_compute(
                            "AllToAll", ALU.bypass,
                            ins=[a2a_in[0][:].opt()], outs=[a2a_out[0][:].opt()],
                            replica_groups=[list(range(NCORES))])

                        # ---- phase 3: batch-1 attention + batch-0 out proj --
                        with tc.tile_pool(name="cstage", bufs=1) as cp, \
                             tc.tile_pool(name="cwork", bufs=2) as cw:
                            wout_sb = cp.tile([128, 8, C], BF16)
                            nc.sync.dma_start(
                                wout_sb[:],
                                wout_d.ap().rearrange("(a p) n -> p a n", p=128))
                            atf = cp.tile([128, 8, SHARD], BF16)
                            nc.sync.dma_start(atf[:, :, 0:512],
                                              a2a_out[0][:].transpose([1, 0, 2]))

                            osts = {}

                            def emit_C(ttk, half, drain_eng):
                                # atomic unit: 16 matmuls in two bank-
                                # alternating half-chains + psum drain
                                if half == 0:
                                    osts[ttk] = cw.tile([128, C], F32,
                                                        name="ostage",
                                                        tag="ostage")
                                ost = osts[ttk]
                                ps_o = pssc.tile([128, 1024], F32, name="ps_o",
                                                 tag="pssc")
                                for cc in range(8):
                                    lhs = atf[:, cc, ttk * 128:(ttk + 1) * 128]
                                    nc.tensor.matmul(
                                        ps_o[:, 0:256], lhsT=lhs,
                                        rhs=wout_sb[:, cc,
                                                    half * 512:half * 512 + 256],
                                        start=(cc == 0), stop=(cc == 7))
                                    nc.tensor.matmul(
                                        ps_o[:, 512:768], lhsT=lhs,
                                        rhs=wout_sb[:, cc, half * 512 + 256:
                                                    half * 512 + 512],
                                        start=(cc == 0), stop=(cc == 7))
                                dst = ost[:, half * 512:(half + 1) * 512] \
                                    .rearrange("p (a b) -> p a b", a=2)
                                src = ps_o[:].rearrange(
                                    "p (a b) -> p a b", a=2)[:, :, 0:256]
                                if drain_eng == "act":
                                    nc.scalar.copy(dst, src)
                                else:
                                    nc.vector.tensor_copy(dst, src)
                                nc.sync.dma_start(
                                    out_d.ap()[ttk * 128:(ttk + 1) * 128,
                                               half * 512:(half + 1) * 512],
                                    ost[:, half * 512:(half + 1) * 512])

                            cunits = [(t, h) for t in range(4) for h in range(2)]

                            for qt in range(NQT):
                                def filler3(kc, qt=qt):
                                    if kc == 16 and qt >= 2 and cunits:
                                        emit_C(*cunits.pop(0), drain_eng="vec")
                                emit_B(1, qt, psav, filler=filler3)

                            nc.gpsimd.collective_compute(
                                "AllToAll", ALU.bypass,
                                ins=[a2a_in[1][:].opt()],
                                outs=[a2a_out[1][:].opt()],
                                replica_groups=[list(range(NCORES))])

                            # leftover batch-0 units overlap the collective
                            while cunits:
                                emit_C(*cunits.pop(0), drain_eng="act")
                            nc.sync.dma_start(atf[:, :, 512:1024],
                                              a2a_out[1][:].transpose([1, 0, 2]))
                            for ttk in range(4, 8):
                                for half in range(2):
                                    emit_C(ttk, half, drain_eng="act")

    nc.compile()
    return nc


def _fold_sin(sin, g):
    out = np.empty_like(sin)
    out[:, :32] = -sin[:, :32] * g[32:]
    out[:, 32:] = sin[:, 32:] * g[:32]
    return out


def kernel(hidden_states, cos, sin, Wqkv, Wout, gq, gk):
    global _LAST_RESULT
    _install_profile_shim()

    hidden_states = np.asarray(hidden_states, dtype=np.float32)
    cos = np.asarray(cos, dtype=np.float32)
    sin = np.asarray(sin, dtype=np.float32)
    Wqkv = np.asarray(Wqkv, dtype=np.float32)
    Wout = np.asarray(Wout, dtype=np.float32)
    gq = np.asarray(gq, dtype=np.float32)
    gk = np.asarray(gk, dtype=np.float32)

    if "nc" not in _CACHE:
        _CACHE["nc"] = _build_graph()
    nc = _CACHE["nc"]

    hsT = np.ascontiguousarray(hidden_states.reshape(TOK, C).T).astype(ml_dtypes.bfloat16)
    cosq = cos * gq[None, :]
    sinq = _fold_sin(sin, gq)
    cosk = cos * gk[None, :]
    sink = _fold_sin(sin, gk)
    trigc = np.concatenate([cosq, cosq, cosk, cosk], axis=1).astype(ml_dtypes.bfloat16)
    trigs = np.concatenate([sinq, sinq, sink, sink], axis=1).astype(ml_dtypes.bfloat16)
    wout_bf = Wout.astype(ml_dtypes.bfloat16)

    in_maps = []
    for c in range(NCORES):
        wq = Wqkv[:, c * 128:(c + 1) * 128]
        wk = Wqkv[:, C + c * 128:C + (c + 1) * 128]
        wv = Wqkv[:, 2 * C + c * 128:2 * C + (c + 1) * 128]
        wqkv_loc = np.ascontiguousarray(
            np.concatenate([wq, wk, wv], axis=1)).astype(ml_dtypes.bfloat16)
        in_maps.append({
            "hsT": hsT, "wqkv": wqkv_loc, "trigc": trigc, "trigs": trigs,
            "wout": wout_bf,
        })

    trace = bool(os.environ.get("BASS_TRACE"))
    res = run_bass_kernel_spmd(nc, in_maps, core_ids=list(range(NCORES)), trace=trace)
    _LAST_RESULT = res

    full = np.empty((B, N, C), dtype=np.float32)
    for c in range(NCORES):
        o = res.results[c]["out"]
        for b in range(B):
            full[b, c * 512:(c + 1) * 512, :] = o[b * 512:(b + 1) * 512]
    return full


# revision 5
# speedup vs baseline: 1.2744x; 1.0022x over previous
"""Trainium2 8-core fused attention kernel (QKV proj + RMSNorm + RoPE + SDPA + out proj).

Sharding: tensor-parallel over heads. Each of the 8 cores computes 2 of the 16
heads end-to-end (QKV projection with its Wqkv column shard, per-head RMSNorm +
RoPE, full softmax attention), then AllToAlls redistribute the per-head
attention outputs so every core holds all 1024 attention channels for 1/8 of
the tokens and applies the full Wout to its token shard.

v3.2: jitter-free attention phases. All stage-A (both batches) runs upfront in
phase 1; the attention loops then carry nothing but scores/exp/AV, with exp
alternating whole-tile between ACT (even kc, exact) and DVE (odd kc,
Schraudolph) so both stay well under the PE's per-iteration budget and the PE
never drops out of its max p-state. Per-qt softmax drains are deferred into
the next q-tile's odd-kc slots (when ACT is exp-idle). The batch-1 AllToAll is
split in two (q-tiles 0-3 fire mid-phase) by remapping output chunk ownership,
shrinking the exposed tail.

Self-contained: hardcodes all shapes from the problem spec.
"""
import os
import sys
import types

import numpy as np
import ml_dtypes

sys.path.insert(0, "/opt/trn_rl_repo")

from concourse import bass, bacc, tile, mybir  # noqa: E402
from concourse.bass_utils import run_bass_kernel_spmd  # noqa: E402
from concourse.masks import make_identity  # noqa: E402

B, N, C, H, D = 2, 4096, 1024, 16, 64
NCORES = 8
TOK = B * N            # 8192 global tokens
NB = N // 128          # 32 token tiles per batch
NMACRO = N // 256      # 16 macro tiles (256 tok) per batch
QTILE = 512
NQT = N // QTILE       # 8 q tiles per batch
KC = N // 128          # 32 key chunks per batch
SHARD = TOK // NCORES  # 1024 tokens per core shard
EPS = 1e-6

F32 = mybir.dt.float32
BF16 = mybir.dt.bfloat16
U16 = mybir.dt.uint16
ALU = mybir.AluOpType
ACTF = mybir.ActivationFunctionType

# Schraudolph exp-via-bits for the DVE: bf16 bitpattern of exp(0.125*s) is
# approx round(A*s + B) as uint16 (error <= +-4.2%, rms 1.8%). Odd key-chunks
# use this (50% of elements); the bias cancels between softmax numerator and
# denominator.
SCHRAUD_A = 16.0 / np.log(2.0)
SCHRAUD_B = 16248.75

_CACHE = {}
_LAST_RESULT = None


def _install_profile_shim():
    """trn_boot skips the NTFF hook when antenv.axon_hooks is missing; supply it."""
    try:
        import antenv
        if getattr(antenv, "axon_hooks", None) is not None:
            return
        from trn_agent_boot.trn_boot import _ntff_profile_via_ctypes
        hook = _ntff_profile_via_ctypes("/opt/axon/libaxon_pjrt.so")
        if hook is None:
            return
        mod = types.ModuleType("antenv.axon_hooks")
        state = {"hook": hook}
        mod.get_axon_ntff_profile_hook = lambda: state["hook"]
        mod.set_axon_ntff_profile_hook = lambda h: state.__setitem__("hook", h)
        sys.modules["antenv.axon_hooks"] = mod
        antenv.axon_hooks = mod
    except Exception:
        pass


def _build_graph():
    nc = bacc.Bacc("TRN2", target_bir_lowering=False, debug=False,
                   enable_asserts=True, num_devices=NCORES)

    hsT_d = nc.dram_tensor("hsT", [C, TOK], BF16, kind="ExternalInput")
    wqkv_d = nc.dram_tensor("wqkv", [C, 384], BF16, kind="ExternalInput")
    trigc_d = nc.dram_tensor("trigc", [N, 256], BF16, kind="ExternalInput")
    trigs_d = nc.dram_tensor("trigs", [N, 256], BF16, kind="ExternalInput")
    wout_d = nc.dram_tensor("wout", [C, C], BF16, kind="ExternalInput")
    out_d = nc.dram_tensor("out", [SHARD, C], F32, kind="ExternalOutput")

    with tile.TileContext(nc) as tc:
        with tc.tile_pool(name="const", bufs=1) as constp, \
             tc.tile_pool(name="dram", bufs=1, space="DRAM") as dram:
            wqkv_sb = constp.tile([128, 8, 384], BF16)
            for cc in range(8):
                nc.sync.dma_start(
                    wqkv_sb[:, cc, :],
                    wqkv_d.ap().rearrange("(a p) n -> p a n", p=128)[:, cc, :])
            ident = constp.tile([128, 128], BF16)
            make_identity(nc, ident[:])

            # b0 uses one AllToAll; b1 is split in two so the first can fire
            # mid-phase. For b1, core 2*q+l receives chunk l (256 tok) of
            # q-tile q (a2a1a: q 0-3) and of q-tile 4+q (a2a1b: q 4-7).
            a2a_in0 = dram.tile([NCORES, 128, SHARD // 2], BF16,
                                name="a2a_in0", tag="a2a_in0")
            a2a_out0 = dram.tile([NCORES, 128, SHARD // 2], BF16,
                                 name="a2a_out0", tag="a2a_out0")
            a2a_in1 = [dram.tile([NCORES, 128, SHARD // 4], BF16,
                                 name=f"a2a_in1{h}", tag=f"a2a_in1{h}")
                       for h in range(2)]
            a2a_out1 = [dram.tile([NCORES, 128, SHARD // 4], BF16,
                                  name=f"a2a_out1{h}", tag=f"a2a_out1{h}")
                        for h in range(2)]

            with tc.tile_pool(name="batch", bufs=1) as bp, \
                 tc.tile_pool(name="work", bufs=3) as wp, \
                 tc.tile_pool(name="probsp", bufs=6) as pp, \
                 tc.tile_pool(name="pssc", bufs=2, space="PSUM") as pssc:

                qT = [bp.tile([128, N], BF16, name=f"qT{b}", tag=f"qT{b}") for b in range(B)]
                kT = [bp.tile([128, N], BF16, name=f"kT{b}", tag=f"kT{b}") for b in range(B)]
                vsb = [bp.tile([128, NB, 2, 65], BF16, name=f"v{b}", tag=f"v{b}")
                       for b in range(B)]
                # atn2[b][h]: col block pq holds q-chunks (2pq, 2pq+1) of head
                # half h transposed
                atn2 = [[bp.tile([128, N // 2], BF16, name=f"at{b}{h}",
                                 tag=f"at{b}{h}") for h in range(2)]
                        for b in range(B)]
                for b in range(B):
                    nc.vector.memset(vsb[b][:, :, :, 64:65], 1.0)

                # PE p-state warmup while the first hsT tiles stream in
                warm = pssc.tile([128, 1024], F32, name="warm", tag="pssc")
                for _ in range(144):
                    nc.tensor.matmul(warm[:, 0:128], lhsT=ident[:], rhs=ident[:],
                                     start=True, stop=True)

                # ---------------- Stage A: QKV + RMSNorm + RoPE ----------------
                def emit_A_load(b, mt, sp):
                    hs_t = []
                    for cc in range(8):
                        t = sp.tile([128, 256], BF16, name=f"hs{cc}", tag=f"hs{cc}")
                        nc.sync.dma_start(
                            t[:], hsT_d.ap()[cc * 128:(cc + 1) * 128,
                                             b * N + mt * 256: b * N + (mt + 1) * 256])
                        hs_t.append(t)
                    trigC = sp.tile([128, 2, 256], BF16, name="trigC", tag="trigC")
                    trigS = sp.tile([128, 2, 256], BF16, name="trigS", tag="trigS")
                    for dst, dt_ in ((trigC, trigc_d), (trigS, trigs_d)):
                        nc.sync.dma_start(
                            dst[:], dt_.ap()[mt * 256:(mt + 1) * 256, :]
                            .rearrange("(s p) d -> p s d", p=128))
                    return hs_t, trigC, trigS

                def emit_A_mt(b, mt, sp, psT):
                    """One 256-token macro tile end to end: QKV (PE), psum
                    drains + squares (ACT), batched rsqrt newton + RoPE (DVE),
                    PE transposes with copies split ACT/DVE."""
                    hs_t, trigC, trigS = emit_A_load(b, mt, sp)
                    qks, sqs = [], []
                    for sub in range(2):
                        tt = mt * 2 + sub
                        ps_qkv = pssc.tile([128, 1024], F32, name="ps_qkv",
                                           tag="pssc")
                        for cc in range(8):
                            lhs = hs_t[cc][:, sub * 128:(sub + 1) * 128]
                            nc.tensor.matmul(
                                ps_qkv[:, 0:256], lhsT=lhs,
                                rhs=wqkv_sb[:, cc, 0:256],
                                start=(cc == 0), stop=(cc == 7))
                            nc.tensor.matmul(
                                ps_qkv[:, 512:640], lhsT=lhs,
                                rhs=wqkv_sb[:, cc, 256:384],
                                start=(cc == 0), stop=(cc == 7))
                        qk_sb = wp.tile([128, 256], BF16, name="qk_sb",
                                        tag="qk_sb", bufs=5)
                        nc.scalar.copy(qk_sb[:], ps_qkv[:, 0:256])
                        nc.scalar.copy(
                            vsb[b][:, tt, :, 0:64],
                            ps_qkv[:, 512:640].rearrange("p (h d) -> p h d", h=2))
                        sq = wp.tile([128, 256], BF16, name="sq", tag="sq", bufs=5)
                        nc.scalar.square(sq[:], qk_sb[:])
                        qks.append(qk_sb)
                        sqs.append(sq)

                    # batched sumsq + newton for both subs: rinv = 8/sqrt(ssq)
                    ssq8 = wp.tile([128, 8], F32, name="ssq8", tag="ssq8", bufs=4)
                    for sub in range(2):
                        nc.vector.tensor_reduce(
                            ssq8[:, sub * 4:(sub + 1) * 4],
                            sqs[sub][:].rearrange("p (a e) -> p a e", a=4),
                            axis=mybir.AxisListType.X, op=ALU.add)
                    yv = wp.tile([128, 8], F32, name="yv", tag="yv", bufs=4)
                    with nc.allow_low_precision(reason="rsqrt newton seed"):
                        nc.vector.tensor_scalar(
                            out=yv[:].bitcast(mybir.dt.int32),
                            in0=ssq8[:].bitcast(mybir.dt.int32),
                            scalar1=1, scalar2=None, op0=ALU.arith_shift_right)
                        nc.vector.tensor_scalar(
                            out=yv[:].bitcast(mybir.dt.int32),
                            in0=yv[:].bitcast(mybir.dt.int32),
                            scalar1=-1, scalar2=0x5F3759DF,
                            op0=ALU.mult, op1=ALU.add)
                    tn = wp.tile([128, 8], F32, name="tn", tag="tn", bufs=4)
                    nc.vector.tensor_mul(tn[:], yv[:], yv[:])
                    nc.vector.tensor_mul(tn[:], tn[:], ssq8[:])
                    nc.vector.tensor_scalar(out=tn[:], in0=tn[:],
                                            scalar1=-4.0, scalar2=12.0,
                                            op0=ALU.mult, op1=ALU.add)
                    nc.vector.tensor_mul(yv[:], yv[:], tn[:])

                    for sub in range(2):
                        tt = mt * 2 + sub
                        qk_sb = qks[sub]
                        # RoPE on unnormalized values (bf16 2x); rinv last
                        d_qk = wp.tile([128, 256], BF16, name="d_qk",
                                       tag="d_qk", bufs=3)
                        nc.vector.tensor_mul(d_qk[:], qk_sb[:], trigC[:, sub, :])
                        trot = wp.tile([128, 256], BF16, name="trot",
                                       tag="trot", bufs=3)
                        v4 = qk_sb[:].rearrange("p (a e) -> p a e", a=8)
                        s4 = trigS[:, sub, :].rearrange("p (a e) -> p a e", a=8)
                        t4 = trot[:].rearrange("p (a e) -> p a e", a=8)
                        nc.vector.tensor_mul(t4[:, 0:8:2, :], v4[:, 1:8:2, :],
                                             s4[:, 0:8:2, :])
                        nc.vector.tensor_mul(t4[:, 1:8:2, :], v4[:, 0:8:2, :],
                                             s4[:, 1:8:2, :])
                        rope = wp.tile([128, 256], BF16, name="rope",
                                       tag="rope", bufs=3)
                        nc.vector.tensor_add(rope[:], d_qk[:], trot[:])
                        d_bf = wp.tile([128, 256], BF16, name="d_bf",
                                       tag="d_bf", bufs=6)
                        nc.vector.tensor_tensor(
                            out=d_bf[:].rearrange("p (a e) -> p a e", a=4),
                            in0=rope[:].rearrange("p (a e) -> p a e", a=4),
                            in1=yv[:, sub * 4:(sub + 1) * 4].unsqueeze(2)
                                .broadcast_to([128, 4, 64]),
                            op=ALU.mult)
                        for half, dst in ((0, qT[b]), (1, kT[b])):
                            ps_t = psT.tile([128, 128], BF16, name="ps_t",
                                            tag="pst")
                            nc.tensor.transpose(
                                ps_t[:], d_bf[:, half * 128:(half + 1) * 128],
                                ident[:])
                            if half == 0:
                                nc.scalar.copy(
                                    dst[:, tt * 128:(tt + 1) * 128], ps_t[:])
                            else:
                                nc.vector.tensor_copy(
                                    dst[:, tt * 128:(tt + 1) * 128], ps_t[:])

                # ---------------- Stage B: attention --------------------------
                # Flipped AV: probs chunk [128kc, 128q] stationary, [v|1] moving.
                # at_ps[:, u, 0:65] (u = hh*4+j) accumulates [128q, 64d | denom].

                def stage_a2a(b, qt, srcblocks):
                    """Stage the 4 transposed 128x128 blocks of (b, qt) into
                    the right a2a DRAM buffer."""
                    for hh in range(2):
                        for lp in range(2):
                            pq = 2 * qt + lp
                            src = atn2[b][hh][:, pq * 128:(pq + 1) * 128]
                            for rh in range(2):
                                blk = src[rh * 64:(rh + 1) * 64, :]
                                if b == 0:
                                    nc.sync.dma_start(
                                        a2a_in0[qt, hh * 64:(hh + 1) * 64,
                                                lp * 256 + rh * 128:
                                                lp * 256 + (rh + 1) * 128],
                                        blk)
                                else:
                                    buf = a2a_in1[0] if qt < 4 else a2a_in1[1]
                                    dest = 2 * (qt % 4) + lp
                                    nc.sync.dma_start(
                                        buf[dest, hh * 64:(hh + 1) * 64,
                                            rh * 128:(rh + 1) * 128],
                                        blk)

                def emit_B(b, qt, psav, filler=None):
                    """Returns deferred drain pieces: 4 normalize pairs + 1
                    transpose/staging piece, to be dripped into the next
                    q-tile's odd-kc slots."""
                    at_ps = psav.tile([128, 8, 128], F32, name="at_ps", tag="psav")
                    prevs = []

                    def av_half(pkc, ppr, hh, stop):
                        for j in range(4):
                            u = hh * 4 + j
                            nc.tensor.matmul(
                                at_ps[:, u, 0:65],
                                lhsT=ppr[:, hh * QTILE + j * 128:
                                         hh * QTILE + (j + 1) * 128],
                                rhs=vsb[b][:, pkc, hh, :],
                                start=(pkc == 0 and j == 0),
                                stop=stop,
                                skip_group_check=(j != 0))

                    def scores(kc, hh):
                        nc.tensor.matmul(
                            ps_s[:, hh * QTILE:(hh + 1) * QTILE],
                            lhsT=kT[b][64 * hh:64 * (hh + 1),
                                       kc * 128:(kc + 1) * 128],
                            rhs=qT[b][64 * hh:64 * (hh + 1),
                                      qt * QTILE:(qt + 1) * QTILE],
                            start=True, stop=True)

                    for kc in range(KC):
                        ps_s = pssc.tile([128, 2 * QTILE], F32, name="ps_s",
                                         tag="pssc")
                        scores(kc, 0)
                        scores(kc, 1)
                        pr = pp.tile([128, 2 * QTILE], BF16, name="pr", tag="pr",
                                     bufs=6)
                        # whole-tile exp, alternating engines
                        if kc % 2 == 0:
                            nc.scalar.activation(pr[:], ps_s[:], ACTF.Exp,
                                                 bias=0.0, scale=0.125)
                        else:
                            with nc.allow_low_precision(reason="schraudolph exp"):
                                nc.vector.tensor_scalar(
                                    out=pr[:].bitcast(U16),
                                    in0=ps_s[:],
                                    scalar1=float(SCHRAUD_A),
                                    scalar2=float(SCHRAUD_B),
                                    op0=ALU.mult, op1=ALU.add)
                        if len(prevs) == 2:
                            ppkc, pppr = prevs.pop(0)
                            av_half(ppkc, pppr, 0, False)
                            av_half(ppkc, pppr, 1, False)
                        if filler is not None:
                            filler(kc)
                        prevs.append((kc, pr))

                    ppkc, pppr = prevs.pop(0)
                    av_half(ppkc, pppr, 0, False)
                    av_half(ppkc, pppr, 1, False)
                    ppkc, pppr = prevs.pop(0)
                    av_half(ppkc, pppr, 0, True)
                    av_half(ppkc, pppr, 1, True)

                    rcp8 = wp.tile([128, 8], F32, name="rcp8", tag="rcp8", bufs=4)
                    nc.vector.reciprocal_approx_fast(
                        out=rcp8[:], in_=at_ps[:, :, 64:65].rearrange(
                            "p u c -> p (u c)"))
                    dsbs = [None] * 4

                    def norm_pair(p):
                        dsbs[p] = wp.tile([128, 128], BF16, name="dsb",
                                          tag="dsb", bufs=8)
                        for u in (2 * p, 2 * p + 1):
                            nc.scalar.activation(
                                dsbs[p][:, (u % 2) * 64:(u % 2) * 64 + 64],
                                at_ps[:, u, 0:64], ACTF.Copy,
                                bias=0.0, scale=rcp8[:, u:u + 1])

                    def trans_stage():
                        for p in range(4):
                            hh, lp = divmod(p, 2)
                            pq = 2 * qt + lp
                            nc.sync.dma_start_transpose(
                                atn2[b][hh][:, pq * 128:(pq + 1) * 128],
                                dsbs[p][:])
                        stage_a2a(b, qt, dsbs)

                    return [lambda p=p: norm_pair(p) for p in range(4)] + \
                        [trans_stage]

                def drain_filler(pieces, extra=None):
                    def filler(kc):
                        if pieces and kc in (1, 3, 5, 7, 9):
                            pieces.pop(0)()
                        if extra is not None:
                            extra(kc)
                    return filler

                # ---- phase 1: ALL stage A (both batches) --------------------
                with tc.tile_pool(name="stream", bufs=6) as sp:
                    with tc.tile_pool(name="psT", bufs=3, space="PSUM") as psT:
                        for b in range(B):
                            for mt in range(NMACRO):
                                emit_A_mt(b, mt, sp, psT)

                # ---- phase 2: batch-0 attention -----------------------------
                with tc.tile_pool(name="psav", bufs=2, space="PSUM") as psav:
                    pend = []
                    for qt in range(NQT):
                        pend = emit_B(0, qt, psav, filler=drain_filler(pend))
                    for pc in pend:  # last q-tile drains inline
                        pc()

                    nc.gpsimd.collective_compute(
                        "AllToAll", ALU.bypass,
                        ins=[a2a_in0[:].opt()], outs=[a2a_out0[:].opt()],
                        replica_groups=[list(range(NCORES))])

                    # ---- phase 3: batch-1 attention + batch-0 out proj ------
                    with tc.tile_pool(name="cstage", bufs=1) as cp, \
                         tc.tile_pool(name="cwork", bufs=2) as cw:
                        wout_sb = cp.tile([128, 8, C], BF16)
                        nc.sync.dma_start(
                            wout_sb[:],
                            wout_d.ap().rearrange("(a p) n -> p a n", p=128))
                        atf = cp.tile([128, 8, SHARD], BF16)
                        nc.sync.dma_start(atf[:, :, 0:512],
                                          a2a_out0[:].transpose([1, 0, 2]))

                        osts = {}

                        def emit_C(ttk, half, drain_eng):
                            if half == 0:
                                osts[ttk] = cw.tile([128, C], F32,
                                                    name="ostage", tag="ostage")
                            ost = osts[ttk]
                            ps_o = pssc.tile([128, 1024], F32, name="ps_o",
                                             tag="pssc")
                            for cc in range(8):
                                lhs = atf[:, cc, ttk * 128:(ttk + 1) * 128]
                                nc.tensor.matmul(
                                    ps_o[:, 0:256], lhsT=lhs,
                                    rhs=wout_sb[:, cc,
                                                half * 512:half * 512 + 256],
                                    start=(cc == 0), stop=(cc == 7))
                                nc.tensor.matmul(
                                    ps_o[:, 512:768], lhsT=lhs,
                                    rhs=wout_sb[:, cc, half * 512 + 256:
                                                half * 512 + 512],
                                    start=(cc == 0), stop=(cc == 7))
                            dst = ost[:, half * 512:(half + 1) * 512] \
                                .rearrange("p (a b) -> p a b", a=2)
                            src = ps_o[:].rearrange(
                                "p (a b) -> p a b", a=2)[:, :, 0:256]
                            if drain_eng == "act":
                                nc.scalar.copy(dst, src)
                            else:
                                nc.vector.tensor_copy(dst, src)
                            nc.sync.dma_start(
                                out_d.ap()[ttk * 128:(ttk + 1) * 128,
                                           half * 512:(half + 1) * 512],
                                ost[:, half * 512:(half + 1) * 512])

                        cunits = [(t, h) for t in range(4) for h in range(2)]

                        pend = []
                        for qt in range(NQT):
                            def extra(kc, qt=qt):
                                if kc == 16 and qt >= 2 and cunits:
                                    emit_C(*cunits.pop(0), drain_eng="vec")
                            pend = emit_B(1, qt, psav,
                                          filler=drain_filler(pend, extra))
                            if qt == 4:
                                # q-tiles 0-3 staged (qt3's drains completed
                                # inside qt4's fillers)
                                nc.gpsimd.collective_compute(
                                    "AllToAll", ALU.bypass,
                                    ins=[a2a_in1[0][:].opt()],
                                    outs=[a2a_out1[0][:].opt()],
                                    replica_groups=[list(range(NCORES))])
                                nc.sync.dma_start(
                                    atf[:, :, 512:768],
                                    a2a_out1[0][:].transpose([1, 0, 2]))
                        for pc in pend:
                            pc()

                        nc.gpsimd.collective_compute(
                            "AllToAll", ALU.bypass,
                            ins=[a2a_in1[1][:].opt()],
                            outs=[a2a_out1[1][:].opt()],
                            replica_groups=[list(range(NCORES))])

                        while cunits:  # leftover b0 units overlap the a2a
                            emit_C(*cunits.pop(0), drain_eng="act")
                        # ttk 4-5 need only the first b1 a2a (landed long ago)
                        for ttk in (4, 5):
                            for half in range(2):
                                emit_C(ttk, half, drain_eng="act")
                        nc.sync.dma_start(atf[:, :, 768:1024],
                                          a2a_out1[1][:].transpose([1, 0, 2]))
                        for ttk in (6, 7):
                            for half in range(2):
                                emit_C(ttk, half, drain_eng="act")

    nc.compile()
    return nc


def _fold_sin(sin, g):
    out = np.empty_like(sin)
    out[:, :32] = -sin[:, :32] * g[32:]
    out[:, 32:] = sin[:, 32:] * g[:32]
    return out


def kernel(hidden_states, cos, sin, Wqkv, Wout, gq, gk):
    global _LAST_RESULT
    _install_profile_shim()

    hidden_states = np.asarray(hidden_states, dtype=np.float32)
    cos = np.asarray(cos, dtype=np.float32)
    sin = np.asarray(sin, dtype=np.float32)
    Wqkv = np.asarray(Wqkv, dtype=np.float32)
    Wout = np.asarray(Wout, dtype=np.float32)
    gq = np.asarray(gq, dtype=np.float32)
    gk = np.asarray(gk, dtype=np.float32)

    if "nc" not in _CACHE:
        _CACHE["nc"] = _build_graph()
    nc = _CACHE["nc"]

    hsT = np.ascontiguousarray(hidden_states.reshape(TOK, C).T).astype(ml_dtypes.bfloat16)
    cosq = cos * gq[None, :]
    sinq = _fold_sin(sin, gq)
    cosk = cos * gk[None, :]
    sink = _fold_sin(sin, gk)
    trigc = np.concatenate([cosq, cosq, cosk, cosk], axis=1).astype(ml_dtypes.bfloat16)
    trigs = np.concatenate([sinq, sinq, sink, sink], axis=1).astype(ml_dtypes.bfloat16)
    wout_bf = Wout.astype(ml_dtypes.bfloat16)

    in_maps = []
    for c in range(NCORES):
        wq = Wqkv[:, c * 128:(c + 1) * 128]
        wk = Wqkv[:, C + c * 128:C + (c + 1) * 128]
        wv = Wqkv[:, 2 * C + c * 128:2 * C + (c + 1) * 128]
        wqkv_loc = np.ascontiguousarray(
            np.concatenate([wq, wk, wv], axis=1)).astype(ml_dtypes.bfloat16)
        in_maps.append({
            "hsT": hsT, "wqkv": wqkv_loc, "trigc": trigc, "trigs": trigs,
            "wout": wout_bf,
        })

    trace = bool(os.environ.get("BASS_TRACE"))
    res = run_bass_kernel_spmd(nc, in_maps, core_ids=list(range(NCORES)), trace=trace)
    _LAST_RESULT = res

    full = np.empty((B, N, C), dtype=np.float32)
    for c in range(NCORES):
        o = res.results[c]["out"]
        # batch 0: plain token sharding
        full[0, c * 512:(c + 1) * 512, :] = o[0:512]
        # batch 1: core 2q+l owns chunk l of q-tiles q and 4+q
        qt, lp = divmod(c, 2)
        full[1, qt * 512 + lp * 256: qt * 512 + (lp + 1) * 256, :] = o[512:768]
        full[1, (4 + qt) * 512 + lp * 256: (4 + qt) * 512 + (lp + 1) * 256, :] = \
            o[768:1024]
    return full
